# revision 33
# baseline (speedup 1.0000x reference)
"""Trainium2 Bass kernel for NeuralGraphHidden (GNN message passing).

Math (per molecule b, atom a):
    deg[b,a]    = #valid edges (edges[b,a,:] != -1)
    summed_atom = atoms[b,a] + sum_s atoms[b, edges[b,a,s]]          (64)
    bond_sum    = sum_s bonds[b,a,s]                                  (8)
    x           = concat(summed_atom, bond_sum)                      (72)
    out[b,a]    = relu(x @ Ws[deg] + bs[deg])  if deg <= 5 else 0   (128)

Design notes (driven by measured TRN2 behaviour on this system):
  * Device-side random-row gathers measured 20-500 ns/row -> the host does
    all *layout* work (degree-sort permutation, neighbour row expansion via
    np.take, bf16 packing, feature-major transposes), which is pure indexed
    data movement; the device does all arithmetic.
  * Everything is delivered FEATURE-MAJOR (partition = feature, free =
    degree-sorted token slot), so the device needs no transposes:
      - xrowsT [112, 15360]: rows 0:64 self atom features, rows 64:112 the
        six raw bond vectors; the bond sum happens inside the matmul because
        Wb is tiled 6x along K in wpack.
      - npairT [128, 23040]: neighbour atom features packed TWO SLOTS PER
        COLUMN (s=2p in partitions 0:64, s=2p+1 in 64:128, zeros when
        2p+1 >= d).  A K=128 matmul against vertically stacked [Wa_d; Wa_d]
        sums both neighbour slots in one pass.  128-partition descriptors
        also load at full DMA rate - 64-partition tiles measured half rate.
      - out[c, tok] = relu(Wd^T x + b) with conv on PARTITIONS, so the bias
        is a per-partition scalar folded into the Scalar-engine relu.
  * Per degree group d (2560 slots), per 512-col quad (one PSUM bank): one
    K=112 main matmul + ceil(d/2) K=128 neighbour-pair matmuls accumulate
    in PSUM; Scalar engine applies bias+relu into bf16.  ~110 instructions.
  * DMA is the roofline (~13.4 MB/core).  ALL transfers ride ONE ring
    (sync) issued in consumption order - a single ring drains FIFO at full
    aggregate bandwidth and preserves arrival order; multi-ring issue
    measured ~40% slower.  Stores interleave after each group-pair.
  * Host unpermutes the sorted output (deg-6 rows are zero).
"""

import sys

sys.path.insert(0, "/opt/trn_rl_repo")

import numpy as np
import ml_dtypes

from contextlib import ExitStack

import concourse.bacc as bacc
import concourse.tile as tile
from concourse import mybir
from concourse.bass_utils import run_bass_kernel_spmd

# Problem shapes (hardcoded per the harness contract).
B, A, D = 1024, 128, 6
F_ATOM, F_BOND, CONV = 64, 8, 128
NCORES = 8
BS = B // NCORES          # molecules per core = 128
T = BS * A                # tokens per core = 16384
ROW = F_ATOM + D * F_BOND               # 112 features per packed row
ROWP = 128                              # padded to 128 partitions: non-128-
                                        # partition DMAs measured ~0.6x rate
GROUP_PAD = 2560                        # per-degree group size (static)
NSORT = D * GROUP_PAD                   # 15360 sorted slots
QW = 512                                # quad width (one PSUM bank of f32)
NQ = GROUP_PAD // QW                    # 5 quads per group
# neighbour pair regions per load chunk (keyed by group): each region is
# [128, GROUP_PAD] with two neighbour slots stacked on the partition dim.
# d5's odd slot s=4 shares a region with d3's s=2 (top/bottom halves) so
# only d1's region carries a zero half.
NREG = {5: 3, 4: 2, 3: 1, 2: 1, 1: 1}   # regions stored per chunk
# matmul plan: group -> list of (chunk, region, part_lo, part_hi)
NPLAN = {
    5: [(5, 0, 0, 128), (5, 1, 0, 128), (5, 2, 0, 64)],
    4: [(4, 0, 0, 128), (4, 1, 0, 128)],
    3: [(3, 0, 0, 128), (5, 2, 64, 128)],
    2: [(2, 0, 0, 128)],
    1: [(1, 0, 0, 128)],
    0: [],
}
PCOL = {}
_off = 0
for _d in (5, 4, 3, 2, 1):
    PCOL[_d] = _off
    _off += NREG[_d] * GROUP_PAD
NPAIR_COLS = _off                       # 8 * 2560 = 20480
DORDER = [0, 5, 4, 3, 2, 1]             # d0 while later loads stream; d1 tail

_f32 = mybir.dt.float32
_bf16 = mybir.dt.bfloat16

_cached = {}


def build_program():
    """Build the (static) per-core Bass/Tile program."""
    nc = bacc.Bacc("TRN2", target_bir_lowering=False, debug=False)

    xrowsT = nc.dram_tensor("xrowsT", [ROWP, NSORT], _bf16,
                            kind="ExternalInput")
    npairT = nc.dram_tensor("npairT", [2 * F_ATOM, NPAIR_COLS], _bf16,
                            kind="ExternalInput")
    wpack = nc.dram_tensor("wpack", [ROWP, D * CONV], _bf16,
                           kind="ExternalInput")
    wstack = nc.dram_tensor("wstack", [2 * F_ATOM, D * CONV], _bf16,
                            kind="ExternalInput")
    bsT = nc.dram_tensor("bsT", [CONV, D], _f32, kind="ExternalInput")
    osortT = nc.dram_tensor("osortT", [CONV, NSORT], _bf16,
                            kind="ExternalOutput")

    with tile.TileContext(nc) as tc, ExitStack() as ctx:
        const_pool = ctx.enter_context(tc.tile_pool(name="const", bufs=1))
        work_pool = ctx.enter_context(tc.tile_pool(name="work", bufs=1))
        ps_pool = ctx.enter_context(tc.tile_pool(name="ps", bufs=8,
                                                 space="PSUM"))

        wp_t = const_pool.tile([ROWP, D * CONV], _bf16, tag="wpack")
        ws_t = const_pool.tile([2 * F_ATOM, D * CONV], _bf16, tag="wstack")
        bs_t = const_pool.tile([CONV, D], _f32, tag="bsT")
        xall = work_pool.tile([ROWP, NSORT], _bf16, tag="xall")
        np_t = {}
        for d in range(1, D):
            np_t[d] = work_pool.tile([2 * F_ATOM, NREG[d] * GROUP_PAD],
                                     _bf16, tag=f"np{d}", name=f"np{d}")
        out_t = {}
        for d in range(D):
            out_t[d] = work_pool.tile([CONV, GROUP_PAD], _bf16,
                                      tag=f"out{d}", name=f"out{d}")

        # ONE ring (sync), consumption order; the two big early tensors are
        # issued first so their descriptor generation (~1 us each on the
        # sequencer) is not delayed behind the tiny weight loads.
        nc.sync.dma_start(out=wp_t[:], in_=wpack[:])
        nc.sync.dma_start(out=ws_t[:], in_=wstack[:])
        nc.sync.dma_start(out=xall[:], in_=xrowsT[:])
        nc.sync.dma_start(
            out=np_t[5][:],
            in_=npairT[:, PCOL[5]:PCOL[5] + NREG[5] * GROUP_PAD])
        nc.sync.dma_start(out=bs_t[:], in_=bsT[:])
        for d in (4, 3, 2, 1):
            nc.sync.dma_start(
                out=np_t[d][:],
                in_=npairT[:, PCOL[d]:PCOL[d] + NREG[d] * GROUP_PAD])

        for d in DORDER:
            xt = xall[:, d * GROUP_PAD:(d + 1) * GROUP_PAD]
            out_g = out_t[d]
            wmain = wp_t[:, d * CONV:(d + 1) * CONV]
            plan = NPLAN[d]
            for q in range(NQ):
                cols = slice(q * QW, (q + 1) * QW)
                ps = ps_pool.tile([CONV, QW], _f32, tag="ps")
                nc.tensor.matmul(out=ps[:], lhsT=wmain, rhs=xt[:, cols],
                                 start=True, stop=(len(plan) == 0))
                for j, (c, r, plo, phi) in enumerate(plan):
                    nc.tensor.matmul(
                        out=ps[:],
                        lhsT=ws_t[plo:phi, d * CONV:(d + 1) * CONV],
                        rhs=np_t[c][plo:phi,
                                    r * GROUP_PAD + cols.start:
                                    r * GROUP_PAD + cols.stop],
                        start=False, stop=(j == len(plan) - 1))
                nc.scalar.activation(out_g[:, cols], ps[:],
                                     mybir.ActivationFunctionType.Relu,
                                     bias=bs_t[:, d:d + 1])
            nc.sync.dma_start(
                out=osortT[:, d * GROUP_PAD:(d + 1) * GROUP_PAD],
                in_=out_g[:])

    nc.compile()
    return nc


def _get_program():
    if "nc" not in _cached:
        _cached["nc"] = build_program()
    return _cached["nc"]


def prep_core_inputs(atoms_s, bonds_s, edges_s, wpack_np, wstack_np, bsT_np):
    """Host-side layout/index prep for one core's shard (numpy only)."""
    deg = (edges_s != -1).sum(axis=-1).reshape(-1)            # [T] natural
    slot_tok = np.full(NSORT, -1, np.int64)   # sorted slot -> natural token
    for d in range(D):
        toks = np.nonzero(deg == d)[0]
        n = len(toks)
        assert n <= GROUP_PAD, f"degree-{d} group has {n} > {GROUP_PAD}"
        slot_tok[d * GROUP_PAD:d * GROUP_PAD + n] = toks

    flat = np.concatenate(
        [atoms_s.reshape(T, F_ATOM), bonds_s.reshape(T, D * F_BOND)], axis=1
    ).astype(ml_dtypes.bfloat16)                              # [T, 112]
    safe = np.maximum(slot_tok, 0)
    xrows = np.where((slot_tok >= 0)[:, None], flat[safe],
                     ml_dtypes.bfloat16(0))                   # [NSORT, 112]
    xrowsT = np.zeros((ROWP, NSORT), ml_dtypes.bfloat16)
    xrowsT[:ROW] = xrows.T                                    # [128, NSORT]

    eflat = edges_s.reshape(T, D)
    bcol = (np.arange(T) // A) * A                            # molecule base
    atoms_flat = flat[:, :F_ATOM]

    def neigh_rows(d, s):
        slots = slot_tok[d * GROUP_PAD:(d + 1) * GROUP_PAD]
        sv = slots >= 0
        st = np.maximum(slots, 0)
        e = np.where(sv, eflat[st, s], -1)
        nat = np.maximum(bcol[st] + e, 0)
        return np.where((e >= 0)[:, None], atoms_flat[nat],
                        ml_dtypes.bfloat16(0))                # [2560, 64]

    zero = np.zeros((GROUP_PAD, F_ATOM), ml_dtypes.bfloat16)
    # region layout must match NREG/NPLAN: chunk d5 = [(5,s0|s1), (5,s2|s3),
    # (5,s4 | 3,s2)], d4 = [(4,s0|s1), (4,s2|s3)], d3 = [(3,s0|s1)],
    # d2 = [(2,s0|s1)], d1 = [(1,s0 | zero)]
    halves = [
        (5, 0, 5, 1), (5, 2, 5, 3), (5, 4, 3, 2),
        (4, 0, 4, 1), (4, 2, 4, 3),
        (3, 0, 3, 1),
        (2, 0, 2, 1),
        (1, 0, None, None),
    ]
    regions = []
    for dlo, slo, dhi, shi in halves:
        lo = neigh_rows(dlo, slo)
        hi = zero if dhi is None else neigh_rows(dhi, shi)
        regions.append(np.concatenate([lo, hi], axis=1))      # [2560, 128]
    npair = np.concatenate(regions, axis=0)                   # [20480, 128]
    npairT = np.ascontiguousarray(npair.T)                    # [128, 20480]

    return {
        "xrowsT": xrowsT,
        "npairT": npairT,
        "wpack": wpack_np,
        "wstack": wstack_np,
        "bsT": bsT_np,
    }, slot_tok


def kernel(atoms, bonds, edges, Ws, bs, trace=False):
    atoms = np.asarray(atoms)
    bonds = np.asarray(bonds)
    edges = np.asarray(edges)
    Ws = np.asarray(Ws)
    bs = np.asarray(bs)

    # wpack[:, d*128:(d+1)*128] = [Wa_d (64) | tile(Wb_d, 6) (48)]; the 6x
    # tiling makes the matmul itself perform the bond sum.
    wfull = np.zeros((D, ROWP, CONV), np.float32)
    wfull[:, :F_ATOM] = Ws[:, :F_ATOM]
    wfull[:, F_ATOM:ROW] = np.tile(Ws[:, F_ATOM:], (1, D, 1))
    wpack_np = np.ascontiguousarray(
        wfull.transpose(1, 0, 2).reshape(ROWP, D * CONV)
    ).astype(ml_dtypes.bfloat16)
    # wstack[:, d*128:(d+1)*128] = [Wa_d; Wa_d] so a K=128 matmul sums a
    # neighbour-slot pair in one pass.
    wstack = np.concatenate([Ws[:, :F_ATOM], Ws[:, :F_ATOM]], axis=1)
    wstack_np = np.ascontiguousarray(
        wstack.transpose(1, 0, 2).reshape(2 * F_ATOM, D * CONV)
    ).astype(ml_dtypes.bfloat16)
    bsT_np = np.ascontiguousarray(bs.T.astype(np.float32))    # [128, 6]

    in_maps, slot_toks = [], []
    for c in range(NCORES):
        sl = slice(c * BS, (c + 1) * BS)
        m, st = prep_core_inputs(atoms[sl], bonds[sl], edges[sl],
                                 wpack_np, wstack_np, bsT_np)
        in_maps.append(m)
        slot_toks.append(st)

    nc = _get_program()
    res = run_bass_kernel_spmd(nc, in_maps, core_ids=list(range(NCORES)),
                               trace=trace)
    kernel.last_results = res

    out = np.zeros((B, A, CONV), np.float32)
    for c in range(NCORES):
        osortT = res.results[c]["osortT"].view(ml_dtypes.bfloat16)
        osort = osortT.reshape(CONV, NSORT).T                 # [NSORT, 128]
        st = slot_toks[c]
        real = st >= 0
        shard = out[c * BS:(c + 1) * BS].reshape(T, CONV)
        shard[st[real]] = osort[real].astype(np.float32)
    return out


# revision 38
# speedup vs baseline: 1.0665x; 1.0665x over previous
"""Trainium2 Bass kernel for NeuralGraphHidden (GNN message passing).

Math (per molecule b, atom a):
    deg[b,a]    = #valid edges (edges[b,a,:] != -1)
    summed_atom = atoms[b,a] + sum_s atoms[b, edges[b,a,s]]          (64)
    bond_sum    = sum_s bonds[b,a,s]                                  (8)
    x           = concat(summed_atom, bond_sum)                      (72)
    out[b,a]    = relu(x @ Ws[deg] + bs[deg])  if deg <= 5 else 0   (128)

Design notes (driven by measured TRN2 behaviour on this system):
  * Device-side random-row gathers measured 20-500 ns/row -> the host does
    all *layout* work (degree-sort permutation, neighbour row expansion via
    np.take, bf16 packing, feature-major transposes), which is pure indexed
    data movement; the device does all arithmetic.
  * Everything is delivered FEATURE-MAJOR (partition = feature, free =
    degree-sorted token slot), so the device needs no transposes:
      - xrowsT [112, 15360]: rows 0:64 self atom features, rows 64:112 the
        six raw bond vectors; the bond sum happens inside the matmul because
        Wb is tiled 6x along K in wpack.
      - npairT [128, 23040]: neighbour atom features packed TWO SLOTS PER
        COLUMN (s=2p in partitions 0:64, s=2p+1 in 64:128, zeros when
        2p+1 >= d).  A K=128 matmul against vertically stacked [Wa_d; Wa_d]
        sums both neighbour slots in one pass.  128-partition descriptors
        also load at full DMA rate - 64-partition tiles measured half rate.
      - out[c, tok] = relu(Wd^T x + b) with conv on PARTITIONS, so the bias
        is a per-partition scalar folded into the Scalar-engine relu.
  * Per degree group d (2560 slots), per 512-col quad (one PSUM bank): one
    K=112 main matmul + ceil(d/2) K=128 neighbour-pair matmuls accumulate
    in PSUM; Scalar engine applies bias+relu into bf16.  ~110 instructions.
  * DMA is the roofline (~13.4 MB/core).  ALL transfers ride ONE ring
    (sync) issued in consumption order - a single ring drains FIFO at full
    aggregate bandwidth and preserves arrival order; multi-ring issue
    measured ~40% slower.  Stores interleave after each group-pair.
  * Host unpermutes the sorted output (deg-6 rows are zero).
"""

import sys

sys.path.insert(0, "/opt/trn_rl_repo")

import numpy as np
import ml_dtypes

from contextlib import ExitStack

import concourse.bacc as bacc
import concourse.tile as tile
from concourse import mybir
from concourse.bass_utils import run_bass_kernel_spmd

# Problem shapes (hardcoded per the harness contract).
B, A, D = 1024, 128, 6
F_ATOM, F_BOND, CONV = 64, 8, 128
NCORES = 8
BS = B // NCORES          # molecules per core = 128
T = BS * A                # tokens per core = 16384
ROW = F_ATOM + D * F_BOND               # 112 features per packed row
ROWP = 128                              # padded to 128 partitions: non-128-
                                        # partition DMAs measured ~0.6x rate
GROUP_PAD = 2560                        # per-degree group size (static)
NSORT = D * GROUP_PAD                   # 15360 sorted slots
QW = 512                                # quad width (one PSUM bank of f32)
NQ = GROUP_PAD // QW                    # 5 quads per group
NPAIR = [(d + 1) // 2 for d in range(D)]    # neighbour s-pairs per group
PCOL = {}
_off = 0
for _d in (5, 4, 3, 2, 1):
    PCOL[_d] = _off
    _off += NPAIR[_d] * GROUP_PAD
NPAIR_COLS = _off                       # 9 * 2560 = 23040
DORDER = [0, 5, 4, 3, 2, 1]             # d0 while later loads stream; d1 tail

_f32 = mybir.dt.float32
_bf16 = mybir.dt.bfloat16

_cached = {}


def build_program():
    """Build the (static) per-core Bass/Tile program."""
    nc = bacc.Bacc("TRN2", target_bir_lowering=False, debug=False)

    xrowsT = nc.dram_tensor("xrowsT", [ROWP, NSORT], _bf16,
                            kind="ExternalInput")
    npairT = nc.dram_tensor("npairT", [2 * F_ATOM, NPAIR_COLS], _bf16,
                            kind="ExternalInput")
    wpack = nc.dram_tensor("wpack", [ROWP, D * CONV], _bf16,
                           kind="ExternalInput")
    wstack = nc.dram_tensor("wstack", [2 * F_ATOM, D * CONV], _bf16,
                            kind="ExternalInput")
    bsT = nc.dram_tensor("bsT", [CONV, D], _f32, kind="ExternalInput")
    osortT = nc.dram_tensor("osortT", [CONV, NSORT], _bf16,
                            kind="ExternalOutput")

    with tile.TileContext(nc) as tc, ExitStack() as ctx:
        const_pool = ctx.enter_context(tc.tile_pool(name="const", bufs=1))
        work_pool = ctx.enter_context(tc.tile_pool(name="work", bufs=1))
        ps_pool = ctx.enter_context(tc.tile_pool(name="ps", bufs=8,
                                                 space="PSUM"))

        wp_t = const_pool.tile([ROWP, D * CONV], _bf16, tag="wpack")
        ws_t = const_pool.tile([2 * F_ATOM, D * CONV], _bf16, tag="wstack")
        bs_t = const_pool.tile([CONV, D], _f32, tag="bsT")
        xall = work_pool.tile([ROWP, NSORT], _bf16, tag="xall")
        np_t = {}
        for d in range(1, D):
            np_t[d] = work_pool.tile([2 * F_ATOM, NPAIR[d] * GROUP_PAD],
                                     _bf16, tag=f"np{d}", name=f"np{d}")
        out_t = {}
        for d in range(D):
            out_t[d] = work_pool.tile([CONV, GROUP_PAD], _bf16,
                                      tag=f"out{d}", name=f"out{d}")

        # ONE ring (sync), consumption order; the two big early tensors are
        # issued first so their descriptor generation (~1 us each on the
        # sequencer) is not delayed behind the tiny weight loads.
        nc.sync.dma_start(out=wp_t[:], in_=wpack[:])
        nc.sync.dma_start(out=ws_t[:], in_=wstack[:])
        nc.sync.dma_start(out=xall[:], in_=xrowsT[:])
        nc.sync.dma_start(
            out=np_t[5][:],
            in_=npairT[:, PCOL[5]:PCOL[5] + NPAIR[5] * GROUP_PAD])
        nc.sync.dma_start(out=bs_t[:], in_=bsT[:])
        for d in (4, 3, 2, 1):
            nc.sync.dma_start(
                out=np_t[d][:],
                in_=npairT[:, PCOL[d]:PCOL[d] + NPAIR[d] * GROUP_PAD])

        for d in DORDER:
            xt = xall[:, d * GROUP_PAD:(d + 1) * GROUP_PAD]
            out_g = out_t[d]
            wmain = wp_t[:, d * CONV:(d + 1) * CONV]
            wpair = ws_t[:, d * CONV:(d + 1) * CONV]
            for q in range(NQ):
                cols = slice(q * QW, (q + 1) * QW)
                ps = ps_pool.tile([CONV, QW], _f32, tag="ps")
                nc.tensor.matmul(out=ps[:], lhsT=wmain, rhs=xt[:, cols],
                                 start=True, stop=(NPAIR[d] == 0))
                for p in range(NPAIR[d]):
                    nc.tensor.matmul(
                        out=ps[:], lhsT=wpair,
                        rhs=np_t[d][:, p * GROUP_PAD + cols.start:
                                    p * GROUP_PAD + cols.stop],
                        start=False, stop=(p == NPAIR[d] - 1))
                nc.scalar.activation(out_g[:, cols], ps[:],
                                     mybir.ActivationFunctionType.Relu,
                                     bias=bs_t[:, d:d + 1])
            nc.sync.dma_start(
                out=osortT[:, d * GROUP_PAD:(d + 1) * GROUP_PAD],
                in_=out_g[:])

    nc.compile()
    return nc


def _get_program():
    if "nc" not in _cached:
        _cached["nc"] = build_program()
    return _cached["nc"]


def prep_core_inputs(atoms_s, bonds_s, edges_s, wpack_np, wstack_np, bsT_np):
    """Host-side layout/index prep for one core's shard (numpy only)."""
    deg = (edges_s != -1).sum(axis=-1).reshape(-1)            # [T] natural
    slot_tok = np.full(NSORT, -1, np.int64)   # sorted slot -> natural token
    for d in range(D):
        toks = np.nonzero(deg == d)[0]
        n = len(toks)
        assert n <= GROUP_PAD, f"degree-{d} group has {n} > {GROUP_PAD}"
        slot_tok[d * GROUP_PAD:d * GROUP_PAD + n] = toks

    flat = np.concatenate(
        [atoms_s.reshape(T, F_ATOM), bonds_s.reshape(T, D * F_BOND)], axis=1
    ).astype(ml_dtypes.bfloat16)                              # [T, 112]
    safe = np.maximum(slot_tok, 0)
    xrows = np.where((slot_tok >= 0)[:, None], flat[safe],
                     ml_dtypes.bfloat16(0))                   # [NSORT, 112]
    xrowsT = np.zeros((ROWP, NSORT), ml_dtypes.bfloat16)
    xrowsT[:ROW] = xrows.T                                    # [128, NSORT]

    eflat = edges_s.reshape(T, D)
    bcol = (np.arange(T) // A) * A                            # molecule base
    atoms_flat = flat[:, :F_ATOM]

    def neigh_rows(d, s):
        slots = slot_tok[d * GROUP_PAD:(d + 1) * GROUP_PAD]
        sv = slots >= 0
        st = np.maximum(slots, 0)
        e = np.where(sv, eflat[st, s], -1)
        nat = np.maximum(bcol[st] + e, 0)
        return np.where((e >= 0)[:, None], atoms_flat[nat],
                        ml_dtypes.bfloat16(0))                # [2560, 64]

    zero = np.zeros((GROUP_PAD, F_ATOM), ml_dtypes.bfloat16)
    regions = []
    for d in (5, 4, 3, 2, 1):                 # chunk order = load order
        for p in range(NPAIR[d]):
            lo = neigh_rows(d, 2 * p)
            hi = neigh_rows(d, 2 * p + 1) if 2 * p + 1 < d else zero
            regions.append(np.concatenate([lo, hi], axis=1))  # [2560, 128]
    npair = np.concatenate(regions, axis=0)                   # [23040, 128]
    npairT = np.ascontiguousarray(npair.T)                    # [128, 23040]

    return {
        "xrowsT": xrowsT,
        "npairT": npairT,
        "wpack": wpack_np,
        "wstack": wstack_np,
        "bsT": bsT_np,
    }, slot_tok


def kernel(atoms, bonds, edges, Ws, bs, trace=False):
    atoms = np.asarray(atoms)
    bonds = np.asarray(bonds)
    edges = np.asarray(edges)
    Ws = np.asarray(Ws)
    bs = np.asarray(bs)

    # wpack[:, d*128:(d+1)*128] = [Wa_d (64) | tile(Wb_d, 6) (48)]; the 6x
    # tiling makes the matmul itself perform the bond sum.
    wfull = np.zeros((D, ROWP, CONV), np.float32)
    wfull[:, :F_ATOM] = Ws[:, :F_ATOM]
    wfull[:, F_ATOM:ROW] = np.tile(Ws[:, F_ATOM:], (1, D, 1))
    wpack_np = np.ascontiguousarray(
        wfull.transpose(1, 0, 2).reshape(ROWP, D * CONV)
    ).astype(ml_dtypes.bfloat16)
    # wstack[:, d*128:(d+1)*128] = [Wa_d; Wa_d] so a K=128 matmul sums a
    # neighbour-slot pair in one pass.
    wstack = np.concatenate([Ws[:, :F_ATOM], Ws[:, :F_ATOM]], axis=1)
    wstack_np = np.ascontiguousarray(
        wstack.transpose(1, 0, 2).reshape(2 * F_ATOM, D * CONV)
    ).astype(ml_dtypes.bfloat16)
    bsT_np = np.ascontiguousarray(bs.T.astype(np.float32))    # [128, 6]

    in_maps, slot_toks = [], []
    for c in range(NCORES):
        sl = slice(c * BS, (c + 1) * BS)
        m, st = prep_core_inputs(atoms[sl], bonds[sl], edges[sl],
                                 wpack_np, wstack_np, bsT_np)
        in_maps.append(m)
        slot_toks.append(st)

    nc = _get_program()
    res = run_bass_kernel_spmd(nc, in_maps, core_ids=list(range(NCORES)),
                               trace=trace)
    kernel.last_results = res

    out = np.zeros((B, A, CONV), np.float32)
    for c in range(NCORES):
        osortT = res.results[c]["osortT"].view(ml_dtypes.bfloat16)
        osort = osortT.reshape(CONV, NSORT).T                 # [NSORT, 128]
        st = slot_toks[c]
        real = st >= 0
        shard = out[c * BS:(c + 1) * BS].reshape(T, CONV)
        shard[st[real]] = osort[real].astype(np.float32)
    return out
